# revision 21
# baseline (speedup 1.0000x reference)
"""Trainium2 Bass kernel for nn_Barrier_Net (DeepSet GNN message passing).

Strategy (8 NeuronCores, SPMD):
  - 128 agent-blocks (128 agents each) are ranked by edge count and dealt
    round-robin to the 8 cores (rank r -> core r%8, slot r//8), so every
    core's slot-j block is no larger than the global rank-8j block: the
    compiled per-slot capacity profile is tight and identical on all cores
    (pure SPMD, no collectives).
  - phi runs transposed (features on partitions, edges free) in 256-edge
    chunks through a software pipeline (one stage per iteration):
    L1 -> relu1 -> L2 -> relu2 -> L3 -> h3-drain -> segsum. Every PSUM
    stage tile is a single bank with a 2-deep ring.
  - Both channel halves of a layer live in one [128, 2, n] PSUM tile and
    are drained by a single wide instruction; bp1 is folded into L1 via a
    ones-row; bp3 and br1 are folded into rho1 via a deg row and a ones
    row appended to the aggregate (rank-1 terms cost nothing: matmul cost
    is output-size-bound). h3 of two consecutive chunks is drained by one
    paired instruction. PSUM->SBUF drains are greedily balanced between
    ACT and DVE.
  - Layer 2 (256x256) runs in fp32r (full fp32 precision at bf16 matmul
    throughput for free size >= 256; fp8 DoubleRow would halve PE time
    but exceeds the accuracy budget).
  - segment-sum is a one-hot matmul; one-hots are built on the otherwise
    idle GpSimd engine (padded edges get id -1 and contribute nothing).
    The last subchunk of a block closes the accumulation group directly.
  - rho runs per PAIR of agent blocks (256 agents, so every fp32r matmul
    keeps free size >= 256) as soon as the pair's aggregates close, with
    pipeline stages spaced 2 emission slots apart so PE never waits on
    the ACT/DVE drains; the final pair is emitted back-to-back to
    minimize the tail.
  - ~20 dummy matmuls on a zeroed scratch tile warm the PE p-state ramp
    while the first edge DMA is in flight.
  - The barrier term and br3 (negligible FLOPs) are added on the host.
"""

import numpy as np

N_AGENTS = 16384
N_EDGES = 524288
N_CORES = 8
AG_PER_CORE = N_AGENTS // N_CORES  # 2048
BLK = 128                          # agents per block
NBLK = AG_PER_CORE // BLK          # 16 blocks (slots) per core
MARGIN = 1.2 * 0.15                # barrier margin
N_WARM = 28                        # PE p-state warmup matmuls: keep PE busy
                                   # through the first DMA wait so the idle
                                   # gap doesn't reset the p-state ramp
                                   # (post-gap work would run 2x slow for
                                   # 3us otherwise)

_compiled = {}


def _build(profile, zero_bp2, zero_br2):
    """Build + schedule the SPMD Bass program. profile[j] = subchunks (128
    edges each) allocated to slot j; slot j on core i holds the global
    agent-block of rank 8*j + i (host deals and un-permutes)."""
    from contextlib import ExitStack

    import concourse.bass as bass
    import concourse.tile as tile
    from concourse import bacc, mybir

    FP = mybir.dt.float32
    RDT = mybir.dt.float32r
    BF = mybir.dt.bfloat16
    slot_esz = [c * 128 for c in profile]       # padded edges per slot
    NSUB = sum(profile)
    assert NSUB % 2 == 0
    # eT streams in fixed even-sized segments so no chunk spans a DMA
    # boundary
    SEG_SUB = 34
    # per-subchunk slot membership and first/last flags
    slot_of, is_first, is_last = [], [], []
    for j in range(NBLK):
        for c in range(profile[j]):
            slot_of.append(j)
            is_first.append(c == 0)
            is_last.append(c == profile[j] - 1)

    nc = bacc.Bacc("TRN2", target_bir_lowering=False, debug=False,
                   num_devices=N_CORES)

    def din(name, shape, dt=None):
        return nc.dram_tensor(name, shape, dt or FP,
                              kind="ExternalInput").ap()

    # cols 0:256 = Wp1 (row 4 = bp1), per-slot edges, 128 trailing scratch
    eT = din("eT", [5, 256 + sum(slot_esz) + 128], RDT)
    # packed [128, x] f32: iota | wr3a | wr3b | bp2a | bp2b | br2a | br2b
    #   | relT
    PK = din("PK", [128, 136 + NSUB])
    PKR = din("PKR", [128, 512], RDT)           # wr2a | wr2b
    W2F = din("W2F", [128, 4, 128], RDT)        # w2a | w2b (k-tile split)
    Wp3 = din("Wp3", [128, 256], BF)            # wp3a | wp3b | ident
    Wr1 = din("Wr1", [66, 256], RDT)            # Wr1 | bp3@Wr1 | br1
    DEGX = din("DEGX", [2, AG_PER_CORE], RDT)   # slot-agent degree | ones
    # out[p, r, s, c] = channel c of agent (2p+s)*128 + r (slot space)
    out_d = nc.dram_tensor("out", [NBLK // 2, 128, 2, 2], FP,
                           kind="ExternalOutput").ap()

    RELU = mybir.ActivationFunctionType.Relu
    COPY = mybir.ActivationFunctionType.Copy
    EQ = mybir.AluOpType.is_equal
    ADD = mybir.AluOpType.add
    MAX = mybir.AluOpType.max

    with tile.TileContext(nc) as tc, ExitStack() as ctx:
        consts = ctx.enter_context(tc.tile_pool(name="consts", bufs=1))
        et_pool = ctx.enter_context(tc.tile_pool(name="et", bufs=2))
        h_pool = ctx.enter_context(tc.tile_pool(name="acts", bufs=6))
        oh_pool = ctx.enter_context(tc.tile_pool(name="oh", bufs=16))
        sm_pool = ctx.enter_context(tc.tile_pool(name="small", bufs=4))
        ps1_pool = ctx.enter_context(
            tc.tile_pool(name="ps1", bufs=2, space="PSUM"))
        ps2_pool = ctx.enter_context(
            tc.tile_pool(name="ps2", bufs=2, space="PSUM"))
        ps3_pool = ctx.enter_context(
            tc.tile_pool(name="ps3", bufs=2, space="PSUM"))
        ps_sm = ctx.enter_context(
            tc.tile_pool(name="ps_sm", bufs=1, space="PSUM"))
        ps_agg = ctx.enter_context(
            tc.tile_pool(name="ps_agg", bufs=1, space="PSUM"))

        def cload(name, ap, dt=FP):
            t = consts.tile(list(ap.shape), dt, tag=name)
            nc.sync.dma_start(t[:], ap)
            return t

        # one DMA covers Wp1 + segment 0's edges so L1 waits a single sem
        w_et0 = consts.tile([5, 256 + SEG_SUB * 128], RDT, tag="w_et0")
        nc.sync.dma_start(w_et0[:], eT[:, 0:256 + SEG_SUB * 128])
        wp1_s = w_et0[:, 0:256]
        etb = [w_et0[:, 256:]]

        # PE p-state warmup: Pool memsets a scratch tile (~300ns), then
        # dummy matmuls keep PE continuously busy through the initial DMA
        # wait; sized to slightly overlap the first real matmul so the PE
        # never goes idle (idle resets pe_busy_start and the next 3us of
        # matmuls run at the mid p-state, ~2.4x slower)
        if N_WARM:
            warm_sb = consts.tile([128, 128], BF, tag="warm_sb")
            nc.gpsimd.memset(warm_sb[:], 0.0)
            warm_ps = ps_agg.tile([128, 512], FP, tag="agg", name="warm_ps")
            for _ in range(N_WARM):
                nc.tensor.matmul(warm_ps[:, 0:128], warm_sb[:], warm_sb[:],
                                 start=True, stop=True)

        # PE p-state warmup: dummy matmuls on a zeroed scratch tile run
        # while the first DMA is in flight, so real work starts at full
        # clock (the cost model ramps over 3us of continuous execution)


        # chunk 0's L1 matmuls are emitted here, before the bulk-const DMAs,
        # so their DMA-sem wait threshold covers only wp1 + segment-0 edges
        pre = {}
        ps1_0 = ps1_pool.tile([128, 2, 256], FP, tag="ps1", name="ps1_0")
        n0 = 256
        nc.tensor.matmul(ps1_0[:, 0, 0:n0], wp1_s[:, 0:128],
                         etb[0][:, 0:n0], start=True, stop=True)
        nc.tensor.matmul(ps1_0[:, 1, 0:n0], wp1_s[:, 128:256],
                         etb[0][:, 0:n0], start=True, stop=True)
        pre[0] = ps1_0
        # const loads ordered by first use: W2F (s2, iter 3), Wp3 (s4,
        # iter 6), PK (s6 one-hots, iter 9), then rho-only consts
        w2f_s = cload("w2f", W2F, dt=RDT)
        w2a_s = w2f_s[:, 0:2, :]
        w2b_s = w2f_s[:, 2:4, :]
        wp3f_s = cload("wp3f", Wp3, dt=BF)
        wp3a_s = wp3f_s[:, 0:64]
        wp3b_s = wp3f_s[:, 64:128]
        identb_s = wp3f_s[:, 128:256]
        pk_s = cload("pk", PK)
        pkr_s = cload("pkr", PKR, dt=RDT)
        wr1_s = cload("wr1", Wr1, dt=RDT)
        iota_s = pk_s[:, 0:128]
        wr3a_s = pk_s[:, 128:130]
        wr3b_s = pk_s[:, 130:132]
        bp2a = pk_s[:, 132:133]
        bp2b = pk_s[:, 133:134]
        br2a = pk_s[:, 134:135]
        br2b = pk_s[:, 135:136]
        relT_s = pk_s[:, 136:]
        wr2a_s = pkr_s[:, 0:256]
        wr2b_s = pkr_s[:, 256:512]
        # aggT rows 0:64 = block aggregates (written per close); row 64 =
        # slot-agent degree, row 65 = ones: bp3 and br1 fold into rho1 as
        # rank-1 terms at zero matmul cost
        aggT_s = consts.tile([66, AG_PER_CORE], RDT, tag="aggT")
        nc.sync.dma_start(aggT_s[64:66, :], DEGX)

        # engine accumulators for greedy drain balancing: 0 = ACT, 1 = DVE
        acc = [0.0, 0.0]

        def drain(out, in_, relu, rows, bias=None):
            """PSUM->SBUF drain on the globally less-loaded engine."""
            ca = (rows + 222) * 0.8333
            cd = (rows + 120) * 1.0417
            e = 0 if acc[0] + ca <= acc[1] + cd else 1
            acc[e] += ca if e == 0 else cd
            if e == 0:
                nc.scalar.activation(out, in_, RELU if relu else COPY,
                                     bias=bias if bias is not None else 0.0)
            elif relu:
                if bias is not None:
                    nc.vector.tensor_scalar(out, in_, bias, 0.0, ADD, MAX)
                else:
                    nc.vector.tensor_scalar(out, in_, 0.0, None, MAX)
            else:
                nc.vector.tensor_copy(out, in_)

        # rho over a PAIR of blocks (256 agents): every fp32r matmul keeps
        # free size >= 256 (below that fp32r runs at 1/4 rate)
        def rho1(p, _prev=None):
            sl = slice(p * 256, (p + 1) * 256)
            pr1 = ps2_pool.tile([128, 2, 256], FP, tag="ps2", name="pr1")
            nc.tensor.matmul(pr1[:, 0, :], wr1_s[:, 0:128],
                             aggT_s[:, sl], start=True, stop=True)
            nc.tensor.matmul(pr1[:, 1, :], wr1_s[:, 128:256],
                             aggT_s[:, sl], start=True, stop=True)
            return pr1

        def rho2(p, pr1):
            r1 = sm_pool.tile([128, 2, 256], RDT, tag="r1")
            drain(r1[:, 0:2, :], pr1[:, 0:2, :], True, 512)
            return r1

        def rho3(p, r1):
            pr2 = ps3_pool.tile([128, 512], FP, tag="ps3", name="pr2")
            for half in (0, 1):
                o = pr2[:, half * 256:(half + 1) * 256]
                nc.tensor.matmul(o, wr2a_s[:, half * 128:(half + 1) * 128],
                                 r1[:, 0, :], start=True, stop=False)
                nc.tensor.matmul(o, wr2b_s[:, half * 128:(half + 1) * 128],
                                 r1[:, 1, :], start=False, stop=True)
            return pr2

        def rho4(p, pr2):
            r2 = sm_pool.tile([128, 512], FP, tag="r2")
            if zero_br2:
                drain(r2[:], pr2[:], True, 512)
            else:
                drain(r2[:, 0:256], pr2[:, 0:256], True, 256,
                      bias=br2a[:, 0:1])
                drain(r2[:, 256:512], pr2[:, 256:512], True, 256,
                      bias=br2b[:, 0:1])
            return r2

        def rho5(p, r2):
            pso = ps_sm.tile([128, 2, 2], FP, tag="sm", name="pso")
            for s in (0, 1):
                nc.tensor.matmul(pso[:, s, :],
                                 r2[:, s * 128:(s + 1) * 128],
                                 wr3a_s[:], start=True, stop=False)
                nc.tensor.matmul(pso[:, s, :],
                                 r2[:, 256 + s * 128:256 + (s + 1) * 128],
                                 wr3b_s[:], start=False, stop=True)
            return pso

        def rho6(p, pso):
            osb = sm_pool.tile([128, 2, 2], FP, tag="osb")
            drain(osb[:], pso[:], False, 4)
            nc.sync.dma_start(out_d[p, :, :, :], osb[:])
            return None

        NCH = NSUB // 2                  # chunks = subchunk pairs
        state = {}

        def s0(k):                       # L1 matmuls (+ segment DMA)
            if k in pre:
                state[("ps1", k)] = pre.pop(k)
                return
            sg, off = divmod(2 * k * 128, SEG_SUB * 128)
            if off == 0 and sg > 0:
                w = 128 * min(SEG_SUB, NSUB - sg * SEG_SUB)
                etb[0] = et_pool.tile([5, SEG_SUB * 128], RDT, tag="etb",
                                      name="etb")
                nc.sync.dma_start(etb[0][:, 0:w],
                                  eT[:, 256 + sg * SEG_SUB * 128:
                                      256 + sg * SEG_SUB * 128 + w])
            ps1 = ps1_pool.tile([128, 2, 256], FP, tag="ps1")
            esl = etb[0][:, off:off + 256]
            nc.tensor.matmul(ps1[:, 0, 0:256], wp1_s[:, 0:128], esl,
                             start=True, stop=True)
            nc.tensor.matmul(ps1[:, 1, 0:256], wp1_s[:, 128:256], esl,
                             start=True, stop=True)
            state[("ps1", k)] = ps1

        def s1(k):                       # relu1 drain -> h1
            n = 256
            ps1 = state.pop(("ps1", k))
            h1 = h_pool.tile([128, 2, 256], RDT, tag="h1")
            drain(h1[:, 0:2, 0:n], ps1[:, 0:2, 0:n], True, 2 * n)
            state[("h1", k)] = h1

        def s2(k):                       # L2 matmuls
            n = 256
            h1 = state.pop(("h1", k))
            ps2 = ps2_pool.tile([128, 2, 256], FP, tag="ps2")
            # complete each half's accumulation group before starting
            # the other: both halves share one PSUM bank and start=True
            # clears the whole bank's has_written bits
            for half, wh in ((0, w2a_s), (1, w2b_s)):
                for kk in (0, 1):
                    nc.tensor.matmul(ps2[:, half, 0:n], wh[:, kk, :],
                                     h1[:, kk, 0:n], start=(kk == 0),
                                     stop=(kk == 1))
            state[("ps2", k)] = ps2

        def s3(k):                       # relu2 drain -> h2
            n = 256
            ps2 = state.pop(("ps2", k))
            h2 = h_pool.tile([128, 2, 256], BF, tag="h2")
            if zero_bp2:
                drain(h2[:, 0:2, 0:n], ps2[:, 0:2, 0:n], True, 2 * n)
            else:
                drain(h2[:, 0, 0:n], ps2[:, 0, 0:n], True, n,
                      bias=bp2a[:, 0:1])
                drain(h2[:, 1, 0:n], ps2[:, 1, 0:n], True, n,
                      bias=bp2b[:, 0:1])
            state[("h2", k)] = h2

        def s4(k):                       # L3 matmuls into paired ps3
            n = 256
            h2 = state.pop(("h2", k))
            if k % 2 == 0:
                state["ps3"] = ps3_pool.tile([128, 512], FP, tag="ps3",
                                             name="ps3")
                state["ps3w"] = 0
            ps3 = state["ps3"]
            base = state["ps3w"]
            nsub = n // 128
            for s in range(nsub):
                sl = slice(s * 128, (s + 1) * 128)
                o3 = slice(base + s * 64, base + (s + 1) * 64)
                nc.tensor.matmul(ps3[:, o3], h2[:, 0, sl], wp3a_s[:],
                                 start=True, stop=False)
                nc.tensor.matmul(ps3[:, o3], h2[:, 1, sl], wp3b_s[:],
                                 start=False, stop=True)
            state[("o3", k)] = (ps3, base, nsub)
            state["ps3w"] = base + nsub * 64

        def s5(k):                       # h3 drain (once per pair)
            if k % 2 == 0 and k + 1 < NCH:
                return                   # drained with its partner
            ps3, base, nsub = state[("o3", k)]
            rows = base + nsub * 64
            h3 = h_pool.tile([128, 512], BF, tag="h3")
            drain(h3[:, 0:rows], ps3[:, 0:rows], False, rows)
            for kk in (k - 1, k) if k % 2 == 1 else (k,):
                p, base, nsub = state.pop(("o3", kk))
                state[("h3", kk)] = (h3, base, nsub)

        def s6(k):                       # one-hot + segsum (+ block close)
            h3, base, nsub = state.pop(("h3", k))
            for s in range(nsub):
                sub = 2 * k + s          # global subchunk index
                j = slot_of[sub]
                if is_first[sub]:
                    state["pagg"] = ps_agg.tile([128, 64], FP, tag="agg",
                                                name="pagg")
                    state["first"] = True
                pagg = state["pagg"]
                oh = oh_pool.tile([128, 128], BF, tag="oh")
                nc.gpsimd.tensor_scalar(oh[:], iota_s[:],
                                        relT_s[:, sub:sub + 1], None, EQ)
                nc.tensor.matmul(pagg[:], oh[:],
                                 h3[:, base + s * 64:base + (s + 1) * 64],
                                 start=state.pop("first", False),
                                 stop=is_last[sub])
                if is_last[sub]:
                    close_slot(j, pagg)

        def close_slot(j, pagg):
            agg_sb = sm_pool.tile([128, 64], BF, tag="aggsb")
            drain(agg_sb[:], pagg[:], False, 64)
            last = (j == NBLK - 1)

            def transp():
                # deferred one emission slot so the agg_sb drain has run
                # before PE reaches the transpose's Ldweights
                pst = ps_sm.tile([64, 128], BF, tag="sm", name="pst")
                nc.tensor.transpose(pst[:], agg_sb[:], identb_s[:])
                drain(aggT_s[0:64, j * 128:(j + 1) * 128], pst[:],
                      False, 128)
            if last:
                transp()
            else:
                pending.append(transp)
            if j % 2 == 1:
                p = j // 2
                # matmul+drain emitted together so the PSUM ring tile is
                # freed promptly; 2 emission slots between groups (and two
                # before the first) so PE never reaches a rho matmul before
                # its input drain ran. The final pair runs compact: during
                # pipeline drain-down PE is idle anyway and the tail is
                # latency-bound.
                groups = [(rho1, rho2), (rho3, rho4), (rho5, rho6)]

                def step(i, prev):
                    out = groups[i][0](p, prev)
                    out = groups[i][1](p, out)
                    if i + 1 < len(groups):
                        if not last:
                            pending.append(lambda: None)
                            pending.append(lambda: None)
                        pending.append(lambda: step(i + 1, out))
                if not last:
                    pending.append(lambda: None)
                    pending.append(lambda: None)
                pending.append(lambda: step(0, None))

        pending = []
        LAGS = [0, 1, 3, 4, 6, 7, 9]     # emission iteration of s0..s6
        phases = [s0, s1, s2, s3, s4, s5, s6]
        for it in range(NCH + LAGS[-1]):
            # emit later stages first so every consumer follows its producer
            for si in range(len(phases) - 1, -1, -1):
                k = it - LAGS[si]
                if 0 <= k < NCH:
                    phases[si](k)
            if 1 <= it <= 5:
                # idle-filler matmuls bridge the W2F const-DMA latency so
                # PE doesn't stall at chunk 0's L2 (~0.9us). Reading etb
                # makes them depend on the edge DMA, so the scheduler can't
                # hoist them into the initial DMA-wait hole.
                wps = ps_agg.tile([128, 512], FP, tag="agg", name="wps")
                nc.tensor.matmul(wps[:], etb[0][:, 0:128],
                                 etb[0][:, 0:512], start=True, stop=True)
            if pending:
                pending.pop(0)()
                if it >= NCH and pending:
                    pending.pop(0)()     # drain-down: PE is idle, compress
        while pending:
            pending.pop(0)()

    nc.compile()
    return nc


def _prep_inputs(edge_feats, segment_ids, ws):
    """Host-side shard + pad. Returns (profile, deal, zero_bp2, zero_br2,
    in_maps)."""
    import ml_dtypes

    seg = np.asarray(segment_ids).astype(np.int64)
    ef = np.asarray(edge_feats, dtype=np.float32)
    bounds = np.searchsorted(seg, np.arange(0, N_AGENTS + 1, BLK))
    counts = np.diff(bounds)                      # edges per 128-agent block
    # deal blocks round-robin by global rank: rank r -> core r%8, slot r//8;
    # slot j's compiled capacity = size of the global rank-8j block
    order = np.argsort(-counts, kind="stable")
    deal = order.reshape(NBLK, N_CORES)           # deal[j, i] = block id
    slotmax = counts[deal[:, 0]]
    profile = [int(np.ceil(c / 128)) for c in slotmax]
    if sum(profile) % 2:
        profile[0] += 1                           # chunks pair subchunks;
    profile = tuple(profile)                      # pad the earliest slot
    slot_esz = [c * 128 for c in profile]
    slot_roff = [sum(profile[:j]) for j in range(NBLK)]
    NSUB = sum(profile)

    zero_bp2 = not np.any(ws["bp2"])
    zero_br2 = not np.any(ws["br2"])
    wp1x = np.concatenate([ws["Wp1"], ws["bp1"].reshape(1, 256)], axis=0)

    # packed [128, 136] head: iota | wr3a | wr3b | bp2a | bp2b | br2a | br2b
    pk_head = np.concatenate([
        np.tile(np.arange(128, dtype=np.float32), (128, 1)),
        ws["Wr3"][0:128, :], ws["Wr3"][128:256, :],
        ws["bp2"][0:128].reshape(128, 1), ws["bp2"][128:256].reshape(128, 1),
        ws["br2"][0:128].reshape(128, 1), ws["br2"][128:256].reshape(128, 1),
    ], axis=1).astype(np.float32)
    pkr = np.ascontiguousarray(np.concatenate(
        [ws["Wr2"][0:128, :], ws["Wr2"][128:256, :]], axis=1),
        dtype=np.float32)
    # rho1 stationary rows 64/65: bp3 and br1 rank-1 folds
    wr1x = np.concatenate([
        ws["Wr1"],
        (ws["bp3"].reshape(1, 64) @ ws["Wr1"]).reshape(1, 256),
        ws["br1"].reshape(1, 256),
    ], axis=0).astype(np.float32)

    const_w = {
        "Wr1": np.ascontiguousarray(wr1x),
        "Wp3": np.ascontiguousarray(np.concatenate(
            [ws["Wp3"][0:128, :], ws["Wp3"][128:256, :],
             np.eye(128, dtype=np.float32)], axis=1)
        ).astype(ml_dtypes.bfloat16),
    }
    # k-tile layout: w2a[k, t, m] = Wp2[t*128 + k, m]
    w2i = ws["Wp2"].reshape(2, 128, 256).transpose(1, 0, 2)
    const_w["W2F"] = np.ascontiguousarray(np.concatenate(
        [w2i[:, :, 0:128], w2i[:, :, 128:256]], axis=1))

    in_maps = []
    for i in range(N_CORES):
        eTt = np.zeros((5, 256 + sum(slot_esz) + 128), np.float32)
        eTt[4, :] = 1.0
        eTt[:, 0:256] = wp1x
        relT = np.full((128, NSUB), -1.0, np.float32)
        deg = np.zeros(AG_PER_CORE, np.float32)
        eoff = 256
        for j in range(NBLK):                    # j = slot
            g = int(deal[j, i])                  # global block in slot j
            s, e = bounds[g], bounds[g + 1]
            cnt = e - s
            eTt[0:4, eoff:eoff + cnt] = ef[s:e].T
            rel = np.full(slot_esz[j], -1.0, np.float32)
            rel[:cnt] = (seg[s:e] - 128 * g).astype(np.float32)
            relT[:, slot_roff[j]:slot_roff[j] + profile[j]] = \
                rel.reshape(profile[j], 128).T
            # deg indexed by SLOT (device addresses slots)
            np.add.at(deg, (seg[s:e] - 128 * g) + 128 * j, 1.0)
            eoff += slot_esz[j]
        degx = np.stack([deg, np.ones(AG_PER_CORE, np.float32)])
        m = {"eT": eTt,
             "DEGX": degx,
             "PKR": pkr,
             "PK": np.concatenate([pk_head, relT], axis=1)}
        m.update(const_w)
        in_maps.append(m)
    return profile, deal, zero_bp2, zero_br2, in_maps


def _host_barrier(edge_feats, segment_ids):
    ef = np.asarray(edge_feats, dtype=np.float64)
    seg = np.asarray(segment_ids).astype(np.int64)
    p = ef[:, :2]
    d = np.sqrt((p * p).sum(1, keepdims=True))
    contrib = -(p / d) / (d - MARGIN)
    barrier = np.zeros((N_AGENTS, 2), np.float64)
    np.add.at(barrier, seg, contrib)
    return barrier


def kernel(edge_feats, segment_ids, Wp1, bp1, Wp2, bp2, Wp3, bp3,
           Wr1, br1, Wr2, br2, Wr3, br3, _trace=False):
    from concourse.bass_utils import run_bass_kernel_spmd

    ws = dict(Wp1=Wp1, bp1=bp1, Wp2=Wp2, bp2=bp2, Wp3=Wp3, bp3=bp3,
              Wr1=Wr1, br1=br1, Wr2=Wr2, br2=br2, Wr3=Wr3, br3=br3)
    ws = {k: np.asarray(v, dtype=np.float32) for k, v in ws.items()}
    profile, deal, zero_bp2, zero_br2, in_maps = _prep_inputs(
        edge_feats, segment_ids, ws)
    key = (profile, zero_bp2, zero_br2)
    if key not in _compiled:
        _compiled[key] = _build(profile, zero_bp2, zero_br2)
    nc = _compiled[key]
    res = run_bass_kernel_spmd(nc, in_maps, list(range(N_CORES)),
                               trace=_trace)
    out = np.empty((N_AGENTS, 2), np.float32)
    for i in range(N_CORES):
        o = res.results[i]["out"]          # [8, 128, 2, 2], pair-major
        for j in range(NBLK):
            g = int(deal[j, i])
            out[g * 128:(g + 1) * 128] = o[j // 2, :, j % 2, :]
    out = (out.astype(np.float64) + _host_barrier(edge_feats, segment_ids)
           + np.asarray(ws["br3"], np.float64).reshape(1, 2))
    if _trace:
        kernel._last_results = res
    return out.astype(np.float32)


# revision 33
# speedup vs baseline: 1.0048x; 1.0048x over previous
"""Trainium2 Bass kernel for nn_Barrier_Net (DeepSet GNN message passing).

Strategy (8 NeuronCores, SPMD):
  - 128 agent-blocks (128 agents each) are ranked by edge count and dealt
    round-robin to the 8 cores (rank r -> core r%8, slot r//8), so every
    core's slot-j block is no larger than the global rank-8j block: the
    compiled per-slot capacity profile is tight and identical on all cores
    (pure SPMD, no collectives).
  - phi runs transposed (features on partitions, edges free) in 256-edge
    chunks through a software pipeline (one stage per iteration):
    L1 -> relu1 -> L2 -> relu2 -> L3 -> h3-drain -> segsum. Every PSUM
    stage tile is a single bank with a 2-deep ring.
  - Both channel halves of a layer live in one [128, 2, n] PSUM tile and
    are drained by a single wide instruction; bp1 is folded into L1 via a
    ones-row; bp3 and br1 are folded into rho1 via a deg row and a ones
    row appended to the aggregate (rank-1 terms cost nothing: matmul cost
    is output-size-bound). h3 of two consecutive chunks is drained by one
    paired instruction. PSUM->SBUF drains are greedily balanced between
    ACT and DVE.
  - Layer 2 (256x256) runs in fp32r (full fp32 precision at bf16 matmul
    throughput for free size >= 256; fp8 DoubleRow would halve PE time
    but exceeds the accuracy budget).
  - segment-sum is a one-hot matmul; one-hots are built on the otherwise
    idle GpSimd engine (padded edges get id -1 and contribute nothing).
    The last subchunk of a block closes the accumulation group directly.
  - rho runs per PAIR of agent blocks (256 agents, so every fp32r matmul
    keeps free size >= 256) as soon as the pair's aggregates close, with
    pipeline stages spaced 2 emission slots apart so PE never waits on
    the ACT/DVE drains; the final pair is emitted back-to-back to
    minimize the tail.
  - ~20 dummy matmuls on a zeroed scratch tile warm the PE p-state ramp
    while the first edge DMA is in flight.
  - The barrier term and br3 (negligible FLOPs) are added on the host.
"""

import numpy as np

N_AGENTS = 16384
N_EDGES = 524288
N_CORES = 8
AG_PER_CORE = N_AGENTS // N_CORES  # 2048
BLK = 128                          # agents per block
NBLK = AG_PER_CORE // BLK          # 16 blocks (slots) per core
MARGIN = 1.2 * 0.15                # barrier margin
N_WARM = 0                         # PE p-state warmup matmuls (measured: a
                                   # warmup bridge regresses, the sim does
                                   # not charge a ramp penalty worth it)

_compiled = {}


def _layout(caps):
    """caps[j] = slot capacity in 64-edge units. Slot j's (padded) edges
    occupy stream positions [cum[j], cum[j+1]); a 128-edge subchunk window
    can straddle a slot boundary, in which case it carries one one-hot
    PORTION per slot. Returns (cap_e, cum, NSUB, portions, NPORT) with
    portions[sub] = [(slot, relT_col, is_first, is_last), ...]."""
    cap_e = [c * 64 for c in caps]
    cum = [0]
    for c in cap_e:
        cum.append(cum[-1] + c)
    total = cum[-1]
    assert total % 256 == 0
    NSUB = total // 128
    portions = []
    col = 0
    for sub in range(NSUB):
        lo, hi = sub * 128, (sub + 1) * 128
        plist = []
        for j in range(NBLK):
            s, e = cum[j], cum[j + 1]
            if s < hi and e > lo:
                plist.append((j, col, s >= lo, e <= hi))
                col += 1
        portions.append(plist)
    return cap_e, cum, NSUB, portions, col


def _build(caps, zero_bp2, zero_br2):
    """Build + schedule the SPMD Bass program. caps[j] = 64-edge units
    allocated to slot j; slot j on core i holds the global agent-block of
    rank 8*j + i (host deals and un-permutes)."""
    from contextlib import ExitStack

    import concourse.bass as bass
    import concourse.tile as tile
    from concourse import bacc, mybir

    FP = mybir.dt.float32
    RDT = mybir.dt.float32r
    BF = mybir.dt.bfloat16
    cap_e, cum, NSUB, portions, NPORT = _layout(caps)
    total_e = cum[-1]
    # eT streams in fixed even-sized segments so no chunk spans a DMA
    # boundary
    SEG_SUB = 34

    nc = bacc.Bacc("TRN2", target_bir_lowering=False, debug=False,
                   num_devices=N_CORES)

    def din(name, shape, dt=None):
        return nc.dram_tensor(name, shape, dt or FP,
                              kind="ExternalInput").ap()

    # cols 0:256 = Wp1 (row 4 = bp1), per-slot edges, 128 trailing scratch
    eT = din("eT", [5, 256 + total_e + 128], RDT)
    # packed [128, x] f32: iota | wr3a | wr3b | bp2a | bp2b | br2a | br2b
    #   | relT (one column per portion)
    PK = din("PK", [128, 136 + NPORT])
    PKR = din("PKR", [128, 512], RDT)           # wr2a | wr2b
    W2F = din("W2F", [128, 4, 128], RDT)        # w2a | w2b (k-tile split)
    Wp3 = din("Wp3", [128, 256], BF)            # wp3a | wp3b | ident
    Wr1 = din("Wr1", [66, 256], RDT)            # Wr1 | bp3@Wr1 | br1
    DEGX = din("DEGX", [2, AG_PER_CORE], RDT)   # slot-agent degree | ones
    # out[p, r, s, c] = channel c of agent (2p+s)*128 + r (slot space)
    out_d = nc.dram_tensor("out", [NBLK // 2, 128, 2, 2], FP,
                           kind="ExternalOutput").ap()

    RELU = mybir.ActivationFunctionType.Relu
    COPY = mybir.ActivationFunctionType.Copy
    EQ = mybir.AluOpType.is_equal
    ADD = mybir.AluOpType.add
    MAX = mybir.AluOpType.max

    with tile.TileContext(nc) as tc, ExitStack() as ctx:
        consts = ctx.enter_context(tc.tile_pool(name="consts", bufs=1))
        et_pool = ctx.enter_context(tc.tile_pool(name="et", bufs=2))
        h_pool = ctx.enter_context(tc.tile_pool(name="acts", bufs=6))
        oh_pool = ctx.enter_context(tc.tile_pool(name="oh", bufs=16))
        sm_pool = ctx.enter_context(tc.tile_pool(name="small", bufs=4))
        ps1_pool = ctx.enter_context(
            tc.tile_pool(name="ps1", bufs=2, space="PSUM"))
        ps2_pool = ctx.enter_context(
            tc.tile_pool(name="ps2", bufs=2, space="PSUM"))
        ps3_pool = ctx.enter_context(
            tc.tile_pool(name="ps3", bufs=2, space="PSUM"))
        # pagg ping-pongs across 2 banks so a straddling window's new-slot
        # matmul never waits on the old slot's agg drain; the tiny pst/pso
        # tiles ride the ps3 ring instead of their own bank
        ps_agg = ctx.enter_context(
            tc.tile_pool(name="ps_agg", bufs=2, space="PSUM"))

        def cload(name, ap, dt=FP):
            t = consts.tile(list(ap.shape), dt, tag=name)
            nc.sync.dma_start(t[:], ap)
            return t

        # one DMA covers Wp1 + segment 0's edges so L1 waits a single sem
        w_et0 = consts.tile([5, 256 + SEG_SUB * 128], RDT, tag="w_et0")
        nc.sync.dma_start(w_et0[:], eT[:, 0:256 + SEG_SUB * 128])
        wp1_s = w_et0[:, 0:256]
        etb = [w_et0[:, 256:]]

        # PE p-state warmup: Pool memsets a scratch tile (~300ns), then
        # dummy matmuls keep PE continuously busy through the initial DMA
        # wait; sized to slightly overlap the first real matmul so the PE
        # never goes idle (idle resets pe_busy_start and the next 3us of
        # matmuls run at the mid p-state, ~2.4x slower)
        if N_WARM:
            warm_sb = consts.tile([128, 128], BF, tag="warm_sb")
            nc.gpsimd.memset(warm_sb[:], 0.0)
            warm_ps = ps_agg.tile([128, 512], FP, tag="agg", name="warm_ps")
            for _ in range(N_WARM):
                nc.tensor.matmul(warm_ps[:, 0:128], warm_sb[:], warm_sb[:],
                                 start=True, stop=True)

        # PE p-state warmup: dummy matmuls on a zeroed scratch tile run
        # while the first DMA is in flight, so real work starts at full
        # clock (the cost model ramps over 3us of continuous execution)


        # chunk 0's L1 matmuls are emitted here, before the bulk-const DMAs,
        # so their DMA-sem wait threshold covers only wp1 + segment-0 edges
        pre = {}
        ps1_0 = ps1_pool.tile([128, 2, 256], FP, tag="ps1", name="ps1_0")
        n0 = 256
        nc.tensor.matmul(ps1_0[:, 0, 0:n0], wp1_s[:, 0:128],
                         etb[0][:, 0:n0], start=True, stop=True)
        nc.tensor.matmul(ps1_0[:, 1, 0:n0], wp1_s[:, 128:256],
                         etb[0][:, 0:n0], start=True, stop=True)
        pre[0] = ps1_0
        # const loads ordered by first use: W2F (s2, iter 3), Wp3 (s4,
        # iter 6), PK (s6 one-hots, iter 9), then rho-only consts
        w2f_s = cload("w2f", W2F, dt=RDT)
        w2a_s = w2f_s[:, 0:2, :]
        w2b_s = w2f_s[:, 2:4, :]
        wp3f_s = cload("wp3f", Wp3, dt=BF)
        wp3a_s = wp3f_s[:, 0:64]
        wp3b_s = wp3f_s[:, 64:128]
        identb_s = wp3f_s[:, 128:256]
        pk_s = cload("pk", PK)
        pkr_s = cload("pkr", PKR, dt=RDT)
        wr1_s = cload("wr1", Wr1, dt=RDT)
        iota_s = pk_s[:, 0:128]
        wr3a_s = pk_s[:, 128:130]
        wr3b_s = pk_s[:, 130:132]
        bp2a = pk_s[:, 132:133]
        bp2b = pk_s[:, 133:134]
        br2a = pk_s[:, 134:135]
        br2b = pk_s[:, 135:136]
        relT_s = pk_s[:, 136:]
        wr2a_s = pkr_s[:, 0:256]
        wr2b_s = pkr_s[:, 256:512]
        # aggT rows 0:64 = block aggregates (written per close); row 64 =
        # slot-agent degree, row 65 = ones: bp3 and br1 fold into rho1 as
        # rank-1 terms at zero matmul cost
        aggT_s = consts.tile([66, AG_PER_CORE], RDT, tag="aggT")
        nc.sync.dma_start(aggT_s[64:66, :], DEGX)

        # engine accumulators for greedy drain balancing: 0 = ACT, 1 = DVE
        acc = [0.0, 0.0]

        def drain(out, in_, relu, rows, bias=None):
            """PSUM->SBUF drain on the globally less-loaded engine."""
            ca = (rows + 222) * 0.8333
            cd = (rows + 120) * 1.0417
            e = 0 if acc[0] + ca <= acc[1] + cd else 1
            acc[e] += ca if e == 0 else cd
            if e == 0:
                nc.scalar.activation(out, in_, RELU if relu else COPY,
                                     bias=bias if bias is not None else 0.0)
            elif relu:
                if bias is not None:
                    nc.vector.tensor_scalar(out, in_, bias, 0.0, ADD, MAX)
                else:
                    nc.vector.tensor_scalar(out, in_, 0.0, None, MAX)
            else:
                nc.vector.tensor_copy(out, in_)

        # rho over a PAIR of blocks (256 agents): every fp32r matmul keeps
        # free size >= 256 (below that fp32r runs at 1/4 rate)
        def rho1(p, _prev=None):
            sl = slice(p * 256, (p + 1) * 256)
            pr1 = ps2_pool.tile([128, 2, 256], FP, tag="ps2", name="pr1")
            nc.tensor.matmul(pr1[:, 0, :], wr1_s[:, 0:128],
                             aggT_s[:, sl], start=True, stop=True)
            nc.tensor.matmul(pr1[:, 1, :], wr1_s[:, 128:256],
                             aggT_s[:, sl], start=True, stop=True)
            return pr1

        def rho2(p, pr1):
            r1 = sm_pool.tile([128, 2, 256], RDT, tag="r1")
            drain(r1[:, 0:2, :], pr1[:, 0:2, :], True, 512)
            return r1

        def rho3(p, r1):
            pr2 = ps3_pool.tile([128, 512], FP, tag="ps3", name="pr2")
            for half in (0, 1):
                o = pr2[:, half * 256:(half + 1) * 256]
                nc.tensor.matmul(o, wr2a_s[:, half * 128:(half + 1) * 128],
                                 r1[:, 0, :], start=True, stop=False)
                nc.tensor.matmul(o, wr2b_s[:, half * 128:(half + 1) * 128],
                                 r1[:, 1, :], start=False, stop=True)
            return pr2

        def rho4(p, pr2):
            r2 = sm_pool.tile([128, 512], FP, tag="r2")
            if zero_br2:
                drain(r2[:], pr2[:], True, 512)
            else:
                drain(r2[:, 0:256], pr2[:, 0:256], True, 256,
                      bias=br2a[:, 0:1])
                drain(r2[:, 256:512], pr2[:, 256:512], True, 256,
                      bias=br2b[:, 0:1])
            return r2

        def rho5(p, r2):
            pso = ps3_pool.tile([128, 2, 2], FP, tag="ps3", name="pso")
            for s in (0, 1):
                nc.tensor.matmul(pso[:, s, :],
                                 r2[:, s * 128:(s + 1) * 128],
                                 wr3a_s[:], start=True, stop=False)
                nc.tensor.matmul(pso[:, s, :],
                                 r2[:, 256 + s * 128:256 + (s + 1) * 128],
                                 wr3b_s[:], start=False, stop=True)
            return pso

        def rho6(p, pso):
            osb = sm_pool.tile([128, 2, 2], FP, tag="osb")
            drain(osb[:], pso[:], False, 4)
            nc.sync.dma_start(out_d[p, :, :, :], osb[:])
            return None

        NCH = NSUB // 2                  # chunks = subchunk pairs
        state = {}

        def s0(k):                       # L1 matmuls (+ segment DMA)
            if k in pre:
                state[("ps1", k)] = pre.pop(k)
                return
            sg, off = divmod(2 * k * 128, SEG_SUB * 128)
            if off == 0 and sg > 0:
                w = min(SEG_SUB * 128, total_e - sg * SEG_SUB * 128)
                etb[0] = et_pool.tile([5, SEG_SUB * 128], RDT, tag="etb",
                                      name="etb")
                nc.sync.dma_start(etb[0][:, 0:w],
                                  eT[:, 256 + sg * SEG_SUB * 128:
                                      256 + sg * SEG_SUB * 128 + w])
            ps1 = ps1_pool.tile([128, 2, 256], FP, tag="ps1")
            esl = etb[0][:, off:off + 256]
            nc.tensor.matmul(ps1[:, 0, 0:256], wp1_s[:, 0:128], esl,
                             start=True, stop=True)
            nc.tensor.matmul(ps1[:, 1, 0:256], wp1_s[:, 128:256], esl,
                             start=True, stop=True)
            state[("ps1", k)] = ps1

        def s1(k):                       # relu1 drain -> h1
            n = 256
            ps1 = state.pop(("ps1", k))
            h1 = h_pool.tile([128, 2, 256], RDT, tag="h1")
            drain(h1[:, 0:2, 0:n], ps1[:, 0:2, 0:n], True, 2 * n)
            state[("h1", k)] = h1

        def s2(k):                       # L2 matmuls
            n = 256
            h1 = state.pop(("h1", k))
            ps2 = ps2_pool.tile([128, 2, 256], FP, tag="ps2")
            # complete each half's accumulation group before starting
            # the other: both halves share one PSUM bank and start=True
            # clears the whole bank's has_written bits
            for half, wh in ((0, w2a_s), (1, w2b_s)):
                for kk in (0, 1):
                    nc.tensor.matmul(ps2[:, half, 0:n], wh[:, kk, :],
                                     h1[:, kk, 0:n], start=(kk == 0),
                                     stop=(kk == 1))
            state[("ps2", k)] = ps2

        def s3(k):                       # relu2 drain -> h2
            n = 256
            ps2 = state.pop(("ps2", k))
            h2 = h_pool.tile([128, 2, 256], BF, tag="h2")
            if zero_bp2:
                drain(h2[:, 0:2, 0:n], ps2[:, 0:2, 0:n], True, 2 * n)
            else:
                drain(h2[:, 0, 0:n], ps2[:, 0, 0:n], True, n,
                      bias=bp2a[:, 0:1])
                drain(h2[:, 1, 0:n], ps2[:, 1, 0:n], True, n,
                      bias=bp2b[:, 0:1])
            state[("h2", k)] = h2

        def s4(k):                       # L3 matmuls into paired ps3
            n = 256
            h2 = state.pop(("h2", k))
            if k % 2 == 0:
                state["ps3"] = ps3_pool.tile([128, 512], FP, tag="ps3",
                                             name="ps3")
                state["ps3w"] = 0
            ps3 = state["ps3"]
            base = state["ps3w"]
            nsub = n // 128
            for s in range(nsub):
                sl = slice(s * 128, (s + 1) * 128)
                o3 = slice(base + s * 64, base + (s + 1) * 64)
                nc.tensor.matmul(ps3[:, o3], h2[:, 0, sl], wp3a_s[:],
                                 start=True, stop=False)
                nc.tensor.matmul(ps3[:, o3], h2[:, 1, sl], wp3b_s[:],
                                 start=False, stop=True)
            state[("o3", k)] = (ps3, base, nsub)
            state["ps3w"] = base + nsub * 64

        def s5(k):                       # h3 drain (once per pair)
            if k % 2 == 0 and k + 1 < NCH:
                return                   # drained with its partner
            ps3, base, nsub = state[("o3", k)]
            rows = base + nsub * 64
            h3 = h_pool.tile([128, 512], BF, tag="h3")
            drain(h3[:, 0:rows], ps3[:, 0:rows], False, rows)
            for kk in (k - 1, k) if k % 2 == 1 else (k,):
                p, base, nsub = state.pop(("o3", kk))
                state[("h3", kk)] = (h3, base, nsub)

        def s6(k):                       # one-hot + segsum (+ block close)
            h3, base, nsub = state.pop(("h3", k))
            for s in range(nsub):
                sub = 2 * k + s          # global subchunk index
                for j, col, first, last in portions[sub]:
                    if first:
                        state["pagg"] = ps_agg.tile([128, 64], FP,
                                                    tag="agg", name="pagg")
                        state["first"] = True
                    pagg = state["pagg"]
                    oh = oh_pool.tile([128, 128], BF, tag="oh")
                    nc.gpsimd.tensor_scalar(oh[:], iota_s[:],
                                            relT_s[:, col:col + 1],
                                            None, EQ)
                    nc.tensor.matmul(
                        pagg[:], oh[:],
                        h3[:, base + s * 64:base + (s + 1) * 64],
                        start=state.pop("first", False), stop=last)
                    if last:
                        close_slot(j, pagg)

        def close_slot(j, pagg):
            agg_sb = sm_pool.tile([128, 64], BF, tag="aggsb")
            drain(agg_sb[:], pagg[:], False, 64)
            last = (j == NBLK - 1)

            def transp():
                # deferred one emission slot so the agg_sb drain has run
                # before PE reaches the transpose's Ldweights
                pst = ps3_pool.tile([64, 128], BF, tag="ps3", name="pst")
                nc.tensor.transpose(pst[:], agg_sb[:], identb_s[:])
                drain(aggT_s[0:64, j * 128:(j + 1) * 128], pst[:],
                      False, 128)
            if last:
                transp()
            else:
                pending.append(transp)
            if j % 2 == 1:
                p = j // 2
                # matmul+drain emitted together so the PSUM ring tile is
                # freed promptly; 2 emission slots between groups (and two
                # before the first) so PE never reaches a rho matmul before
                # its input drain ran. The final pair runs compact: during
                # pipeline drain-down PE is idle anyway and the tail is
                # latency-bound.
                groups = [(rho1, rho2), (rho3, rho4), (rho5, rho6)]

                def step(i, prev):
                    out = groups[i][0](p, prev)
                    out = groups[i][1](p, out)
                    if i + 1 < len(groups):
                        if not last:
                            pending.append(lambda: None)
                            pending.append(lambda: None)
                        pending.append(lambda: step(i + 1, out))
                if not last:
                    pending.append(lambda: None)
                    pending.append(lambda: None)
                pending.append(lambda: step(0, None))

        pending = []
        LAGS = [0, 1, 3, 4, 6, 7, 9]     # emission iteration of s0..s6
        phases = [s0, s1, s2, s3, s4, s5, s6]
        for it in range(NCH + LAGS[-1]):
            # emit later stages first so every consumer follows its producer
            for si in range(len(phases) - 1, -1, -1):
                k = it - LAGS[si]
                if 0 <= k < NCH:
                    phases[si](k)
            if 1 <= it <= 5:
                # idle-filler matmuls bridge the W2F const-DMA latency so
                # PE doesn't stall at chunk 0's L2 (~0.9us). Reading etb
                # makes them depend on the edge DMA, so the scheduler can't
                # hoist them into the initial DMA-wait hole.
                wps = ps_agg.tile([128, 512], FP, tag="agg", name="wps")
                nc.tensor.matmul(wps[:], etb[0][:, 0:128],
                                 etb[0][:, 0:512], start=True, stop=True)
            if pending:
                pending.pop(0)()
                if it >= NCH and pending:
                    pending.pop(0)()     # drain-down: PE is idle, compress
        while pending:
            pending.pop(0)()

    nc.compile()
    return nc


def _prep_inputs(edge_feats, segment_ids, ws):
    """Host-side shard + pad. Returns (profile, deal, zero_bp2, zero_br2,
    in_maps)."""
    import ml_dtypes

    seg = np.asarray(segment_ids).astype(np.int64)
    ef = np.asarray(edge_feats, dtype=np.float32)
    bounds = np.searchsorted(seg, np.arange(0, N_AGENTS + 1, BLK))
    counts = np.diff(bounds)                      # edges per 128-agent block
    # deal blocks round-robin by global rank: rank r -> core r%8, slot r//8;
    # slot j's compiled capacity = size of the global rank-8j block,
    # rounded up to 64-edge units (subchunk windows may straddle slots)
    order = np.argsort(-counts, kind="stable")
    deal = order.reshape(NBLK, N_CORES)           # deal[j, i] = block id
    slotmax = counts[deal[:, 0]]
    caps = [int(np.ceil(c / 64)) for c in slotmax]
    if sum(caps) % 4:
        caps[0] += 4 - sum(caps) % 4              # chunks pair subchunks;
    caps = tuple(caps)                            # pad the earliest slot
    cap_e, cum, NSUB, portions, NPORT = _layout(caps)
    total_e = cum[-1]

    zero_bp2 = not np.any(ws["bp2"])
    zero_br2 = not np.any(ws["br2"])
    wp1x = np.concatenate([ws["Wp1"], ws["bp1"].reshape(1, 256)], axis=0)

    # packed [128, 136] head: iota | wr3a | wr3b | bp2a | bp2b | br2a | br2b
    pk_head = np.concatenate([
        np.tile(np.arange(128, dtype=np.float32), (128, 1)),
        ws["Wr3"][0:128, :], ws["Wr3"][128:256, :],
        ws["bp2"][0:128].reshape(128, 1), ws["bp2"][128:256].reshape(128, 1),
        ws["br2"][0:128].reshape(128, 1), ws["br2"][128:256].reshape(128, 1),
    ], axis=1).astype(np.float32)
    pkr = np.ascontiguousarray(np.concatenate(
        [ws["Wr2"][0:128, :], ws["Wr2"][128:256, :]], axis=1),
        dtype=np.float32)
    # rho1 stationary rows 64/65: bp3 and br1 rank-1 folds
    wr1x = np.concatenate([
        ws["Wr1"],
        (ws["bp3"].reshape(1, 64) @ ws["Wr1"]).reshape(1, 256),
        ws["br1"].reshape(1, 256),
    ], axis=0).astype(np.float32)

    const_w = {
        "Wr1": np.ascontiguousarray(wr1x),
        "Wp3": np.ascontiguousarray(np.concatenate(
            [ws["Wp3"][0:128, :], ws["Wp3"][128:256, :],
             np.eye(128, dtype=np.float32)], axis=1)
        ).astype(ml_dtypes.bfloat16),
    }
    # k-tile layout: w2a[k, t, m] = Wp2[t*128 + k, m]
    w2i = ws["Wp2"].reshape(2, 128, 256).transpose(1, 0, 2)
    const_w["W2F"] = np.ascontiguousarray(np.concatenate(
        [w2i[:, :, 0:128], w2i[:, :, 128:256]], axis=1))

    in_maps = []
    for i in range(N_CORES):
        eTt = np.zeros((5, 256 + total_e + 128), np.float32)
        eTt[4, :] = 1.0
        eTt[:, 0:256] = wp1x
        stream_rel = np.full(total_e, -1.0, np.float32)
        stream_slot = np.full(total_e, -1, np.int64)
        deg = np.zeros(AG_PER_CORE, np.float32)
        for j in range(NBLK):                    # j = slot
            g = int(deal[j, i])                  # global block in slot j
            s, e = bounds[g], bounds[g + 1]
            cnt = e - s
            eTt[0:4, 256 + cum[j]:256 + cum[j] + cnt] = ef[s:e].T
            stream_rel[cum[j]:cum[j] + cnt] = \
                (seg[s:e] - 128 * g).astype(np.float32)
            stream_slot[cum[j]:cum[j + 1]] = j
            # deg indexed by SLOT (device addresses slots)
            np.add.at(deg, (seg[s:e] - 128 * g) + 128 * j, 1.0)
        relT = np.full((128, NPORT), -1.0, np.float32)
        for sub, plist in enumerate(portions):
            wr = stream_rel[sub * 128:(sub + 1) * 128]
            wsl = stream_slot[sub * 128:(sub + 1) * 128]
            for j, col, first, last in plist:
                relT[:, col] = np.where(wsl == j, wr, -1.0)
        degx = np.stack([deg, np.ones(AG_PER_CORE, np.float32)])
        m = {"eT": eTt,
             "DEGX": degx,
             "PKR": pkr,
             "PK": np.concatenate([pk_head, relT], axis=1)}
        m.update(const_w)
        in_maps.append(m)
    return caps, deal, zero_bp2, zero_br2, in_maps


def _host_barrier(edge_feats, segment_ids):
    ef = np.asarray(edge_feats, dtype=np.float64)
    seg = np.asarray(segment_ids).astype(np.int64)
    p = ef[:, :2]
    d = np.sqrt((p * p).sum(1, keepdims=True))
    contrib = -(p / d) / (d - MARGIN)
    barrier = np.zeros((N_AGENTS, 2), np.float64)
    np.add.at(barrier, seg, contrib)
    return barrier


def kernel(edge_feats, segment_ids, Wp1, bp1, Wp2, bp2, Wp3, bp3,
           Wr1, br1, Wr2, br2, Wr3, br3, _trace=False):
    from concourse.bass_utils import run_bass_kernel_spmd

    ws = dict(Wp1=Wp1, bp1=bp1, Wp2=Wp2, bp2=bp2, Wp3=Wp3, bp3=bp3,
              Wr1=Wr1, br1=br1, Wr2=Wr2, br2=br2, Wr3=Wr3, br3=br3)
    ws = {k: np.asarray(v, dtype=np.float32) for k, v in ws.items()}
    caps, deal, zero_bp2, zero_br2, in_maps = _prep_inputs(
        edge_feats, segment_ids, ws)
    key = (caps, zero_bp2, zero_br2)
    if key not in _compiled:
        _compiled[key] = _build(caps, zero_bp2, zero_br2)
    nc = _compiled[key]
    res = run_bass_kernel_spmd(nc, in_maps, list(range(N_CORES)),
                               trace=_trace)
    out = np.empty((N_AGENTS, 2), np.float32)
    for i in range(N_CORES):
        o = res.results[i]["out"]          # [8, 128, 2, 2], pair-major
        for j in range(NBLK):
            g = int(deal[j, i])
            out[g * 128:(g + 1) * 128] = o[j // 2, :, j % 2, :]
    out = (out.astype(np.float64) + _host_barrier(edge_feats, segment_ids)
           + np.asarray(ws["br3"], np.float64).reshape(1, 2))
    if _trace:
        kernel._last_results = res
    return out.astype(np.float32)
